# revision 6
# baseline (speedup 1.0000x reference)
"""Trainium2 Bass kernel for nn_NCFG_21139829031662 (gnn_message_passing).

RippleNet-style model: hop-0 seed-set sum + 2 hops of (gather triples,
attention softmax over K, 2-step tanh RNN, weighted sum), then a
user/item dot + sigmoid.

Strategy: pure data-parallel over the 4096-user batch across 8 cores
(512 users/core); embedding tables replicated in each core's HBM.
The dominant cost is ~1.3M random 128B gathers from the 64MB entity
table via SWDGE indirect DMA. HW semantics (probed): each
indirect_dma_start consumes exactly ONE offset per destination
partition, so a call fetches at most 128 random rows; the ~1us
fixed SWDGE cost per call makes the gather stream Pool-engine-serial.

Call-count reductions vs the naive layout:
  - relations: 64-row table -> host-built triple-product table
    [64^3, 96] so one offset fetches 3 j-columns of relation rows
    (512 -> 176 calls per core).
  - final item gathers: host-fused entity||rec_item table [200K, 64]
    so one offset fetches both embeddings (32 -> 16 calls).
  - all index tensors preloaded in one burst (no per-batch idx DMAs).

Per-core on-chip layout ("G-layout"): token (u, k) -> partition
p = (u%2)*64 + k, free column j = u//2 (32 f32 per column). This makes:
  - softmax over K a per-j-column partition-group sum (done on PE with
    0/1 selector matmuls),
  - the RNN matmuls feature-major via PE transposes of [128,128] blocks
    with 4-way block-diagonal weights,
  - the weighted hop reduction a PE selector matmul accumulating into a
    single [32, 512] PSUM bank across all hops.
"""

import sys
import numpy as np

sys.path.insert(0, "/opt/trn_rl_repo")

# ---------------------------------------------------------------- constants
DIM = 32
N_ENTITY = 500000
N_RELATION = 64
N_USER = 100000
N_ITEM = 200000
B = 4096
K = 64
L = 2
NCORES = 8
P = 128

NREL3 = N_RELATION ** 3   # triple-product relation table rows
TRI = 10                  # triples per 32-column batch (30 cols)
# cols 30,31 of each batch go through the pair table
NRELP = N_RELATION ** 2


def build_core_program(BC=512, JB=32):
    """Build the single-core bass program (SPMD: same program on all cores).

    BC: users per core. JB: j-columns (user pairs) per processing batch.
    """
    import concourse.bass as bass
    import concourse.bacc as bacc
    import concourse.mybir as mybir
    import concourse.tile as tile
    from concourse.masks import make_identity

    J = BC // 2              # j-columns total
    NBATCH = J // JB         # batches per hop
    NCHUNK = J // 16         # 16-j output chunks
    NR = 2 * NCHUNK          # output psum rows
    assert J % JB == 0 and JB % 16 == 0
    CPB = JB // 16           # chunks per batch
    STB = JB // 4            # supertiles ([128,128] blocks) per batch
    f32 = mybir.dt.float32
    i32 = mybir.dt.int32

    nc = bacc.Bacc("TRN2", target_bir_lowering=False, debug=False)

    # DRAM inputs
    entity = nc.dram_tensor("entity", [N_ENTITY, DIM], f32, kind="ExternalInput").ap()
    rel3 = nc.dram_tensor("rel3", [NREL3, 3 * DIM], f32, kind="ExternalInput").ap()
    relp = nc.dram_tensor("relp", [NRELP, 2 * DIM], f32, kind="ExternalInput").ap()
    rec_user = nc.dram_tensor("rec_user", [N_USER, DIM], f32, kind="ExternalInput").ap()
    itab = nc.dram_tensor("itab", [N_ITEM, 2 * DIM], f32, kind="ExternalInput").ap()
    idx_hop0 = nc.dram_tensor("idx_hop0", [P, J], i32, kind="ExternalInput").ap()
    idx_h = nc.dram_tensor("idx_h", [P, L * J], i32, kind="ExternalInput").ap()
    idx_t = nc.dram_tensor("idx_t", [P, L * J], i32, kind="ExternalInput").ap()
    idx_r3 = nc.dram_tensor("idx_r3", [P, L * NBATCH * TRI], i32, kind="ExternalInput").ap()
    idx_rp = nc.dram_tensor("idx_rp", [P, L * NBATCH], i32, kind="ExternalInput").ap()
    fin_users = nc.dram_tensor("fin_users", [NR, 16], i32, kind="ExternalInput").ap()
    fin_items = nc.dram_tensor("fin_items", [NR, 16], i32, kind="ExternalInput").ap()
    wh_bd = nc.dram_tensor("wh_bd", [P, P], f32, kind="ExternalInput").ap()
    wr_bd = nc.dram_tensor("wr_bd", [P, P], f32, kind="ExternalInput").ap()
    whh_bd = nc.dram_tensor("whh_bd", [P, P], f32, kind="ExternalInput").ap()
    b2_in = nc.dram_tensor("b2", [P, 1], f32, kind="ExternalInput").ap()
    sels_in = nc.dram_tensor("sels", [P, NCHUNK * NR], f32, kind="ExternalInput").ap()
    par2_in = nc.dram_tensor("par2", [P, 2], f32, kind="ExternalInput").ap()
    parT_in = nc.dram_tensor("parT", [2, P], f32, kind="ExternalInput").ap()
    out_dram = nc.dram_tensor("scores", [NR, 16], f32, kind="ExternalOutput").ap()

    with tile.TileContext(nc) as tc:
        with (
            tc.tile_pool(name="const", bufs=1) as cpool,
            tc.tile_pool(name="gath", bufs=3) as gpool,
            tc.tile_pool(name="work", bufs=2) as wpool,
            tc.tile_pool(name="small", bufs=2) as spool,
            tc.tile_pool(name="psO", bufs=1, space="PSUM") as poolO,
            tc.tile_pool(name="psT", bufs=2, space="PSUM") as poolT,
            tc.tile_pool(name="psR", bufs=1, space="PSUM") as poolR,
            tc.tile_pool(name="psS", bufs=1, space="PSUM") as poolS,
        ):
            # ---------------- constants + all index tensors to SBUF upfront
            ident = cpool.tile([P, P], f32, tag="ident")
            make_identity(nc, ident[:])
            wh_t = cpool.tile([P, P], f32, tag="wh")
            nc.sync.dma_start(out=wh_t[:], in_=wh_bd[:, :])
            wr_t = cpool.tile([P, P], f32, tag="wr")
            nc.sync.dma_start(out=wr_t[:], in_=wr_bd[:, :])
            whh_t = cpool.tile([P, P], f32, tag="whh")
            nc.sync.dma_start(out=whh_t[:], in_=whh_bd[:, :])
            b2_t = cpool.tile([P, 1], f32, tag="b2")
            nc.sync.dma_start(out=b2_t[:], in_=b2_in[:, :])
            sels_t = cpool.tile([P, NCHUNK * NR], f32, tag="sels")
            nc.sync.dma_start(out=sels_t[:], in_=sels_in[:, :])
            par2_t = cpool.tile([P, 2], f32, tag="par2")
            nc.sync.dma_start(out=par2_t[:], in_=par2_in[:, :])
            parT_t = cpool.tile([2, P], f32, tag="parT")
            nc.sync.dma_start(out=parT_t[:], in_=parT_in[:, :])

            i0_t = cpool.tile([P, J], i32, tag="i0")
            nc.sync.dma_start(out=i0_t[:], in_=idx_hop0[:, :])
            ih_t = cpool.tile([P, L * J], i32, tag="ih")
            nc.sync.dma_start(out=ih_t[:], in_=idx_h[:, :])
            it_t = cpool.tile([P, L * J], i32, tag="it")
            nc.sync.dma_start(out=it_t[:], in_=idx_t[:, :])
            ir3_t = cpool.tile([P, L * NBATCH * TRI], i32, tag="ir3")
            nc.sync.dma_start(out=ir3_t[:], in_=idx_r3[:, :])
            irp_t = cpool.tile([P, L * NBATCH], i32, tag="irp")
            nc.sync.dma_start(out=irp_t[:], in_=idx_rp[:, :])
            fu_t = cpool.tile([NR, 16], i32, tag="fu")
            nc.sync.dma_start(out=fu_t[:], in_=fin_users[:, :])
            fi_t = cpool.tile([NR, 16], i32, tag="fi")
            nc.sync.dma_start(out=fi_t[:], in_=fin_items[:, :])

            # persistent output accumulator [NR, 512] (one PSUM bank)
            o_ps = poolO.tile([NR, 512], f32, tag="o")
            first_omm = [True]

            def o_accum(rhs_ap, chunk, is_last):
                """rhs [128, 512] -> accumulate selector chunk into o_ps."""
                nc.tensor.matmul(
                    out=o_ps[:, :],
                    lhsT=sels_t[:, chunk * NR:(chunk + 1) * NR],
                    rhs=rhs_ap,
                    start=first_omm[0],
                    stop=is_last,
                    skip_group_check=True,
                )
                first_omm[0] = False

            # ---------------- hop 0: gather + selector-sum
            for b in range(NBATCH):
                g0 = gpool.tile([P, JB * DIM], f32, tag="h")
                for jj in range(JB):
                    c = b * JB + jj
                    nc.gpsimd.indirect_dma_start(
                        out=g0[:, jj * DIM:(jj + 1) * DIM],
                        out_offset=None,
                        in_=entity[:, :],
                        in_offset=bass.IndirectOffsetOnAxis(
                            ap=i0_t[:, c:c + 1], axis=0),
                    )
                for c in range(CPB):
                    o_accum(g0[:, c * 512:(c + 1) * 512], b * CPB + c, False)

            # ---------------- hops
            for l in range(L):
                for b in range(NBATCH):
                    jlo = b * JB
                    # gathers: one [128,1]-offset indirect DMA per j-column
                    # for entity h/t; relation rows come 3-at-a-time from the
                    # triple table (+ one pair call for the last 2 columns).
                    Hg = gpool.tile([P, JB * DIM], f32, tag="h")
                    Rg = gpool.tile([P, JB * DIM], f32, tag="r")
                    Tg = gpool.tile([P, JB * DIM], f32, tag="t")
                    for jj in range(JB):
                        sl = slice(jj * DIM, (jj + 1) * DIM)
                        c = l * J + jlo + jj
                        nc.gpsimd.indirect_dma_start(
                            out=Hg[:, sl], out_offset=None, in_=entity[:, :],
                            in_offset=bass.IndirectOffsetOnAxis(
                                ap=ih_t[:, c:c + 1], axis=0))
                        nc.gpsimd.indirect_dma_start(
                            out=Tg[:, sl], out_offset=None, in_=entity[:, :],
                            in_offset=bass.IndirectOffsetOnAxis(
                                ap=it_t[:, c:c + 1], axis=0))
                    for jt in range(TRI):
                        c3 = (l * NBATCH + b) * TRI + jt
                        nc.gpsimd.indirect_dma_start(
                            out=Rg[:, jt * 96:(jt + 1) * 96],
                            out_offset=None, in_=rel3[:, :],
                            in_offset=bass.IndirectOffsetOnAxis(
                                ap=ir3_t[:, c3:c3 + 1], axis=0))
                    cp = l * NBATCH + b
                    nc.gpsimd.indirect_dma_start(
                        out=Rg[:, TRI * 96:TRI * 96 + 64],
                        out_offset=None, in_=relp[:, :],
                        in_offset=bass.IndirectOffsetOnAxis(
                            ap=irp_t[:, cp:cp + 1], axis=0))

                    # ---- logits: d = sum_d h*t + sum_d r*r ; pi = softmax_k
                    prod = wpool.tile([P, JB * DIM], f32, tag="prod")
                    nc.vector.tensor_tensor(
                        out=prod[:], in0=Hg[:], in1=Tg[:], op=mybir.AluOpType.mult)
                    dht = spool.tile([P, JB], f32, tag="dht")
                    nc.vector.tensor_reduce(
                        out=dht[:], in_=prod[:].rearrange("p (j d) -> p j d", d=DIM),
                        axis=mybir.AxisListType.X, op=mybir.AluOpType.add)
                    sq = wpool.tile([P, JB * DIM], f32, tag="sq")
                    nc.scalar.square(out=sq[:], in_=Rg[:])
                    drr = spool.tile([P, JB], f32, tag="drr")
                    nc.vector.tensor_reduce(
                        out=drr[:], in_=sq[:].rearrange("p (j d) -> p j d", d=DIM),
                        axis=mybir.AxisListType.X, op=mybir.AluOpType.add)
                    logits = spool.tile([P, JB], f32, tag="lg")
                    nc.vector.tensor_tensor(
                        out=logits[:], in0=dht[:], in1=drr[:], op=mybir.AluOpType.add)
                    E = spool.tile([P, JB], f32, tag="E")
                    nc.scalar.activation(
                        out=E[:], in_=logits[:], func=mybir.ActivationFunctionType.Exp)
                    # denominators: [2, JB] = parity sums of E
                    den_ps = poolS.tile([2, JB], f32, tag="dn")
                    nc.tensor.matmul(out=den_ps[:], lhsT=par2_t[:], rhs=E[:],
                                     start=True, stop=True)
                    rec = spool.tile([2, JB], f32, tag="rec")
                    nc.vector.reciprocal(out=rec[:], in_=den_ps[:])
                    rb_ps = poolS.tile([P, JB], f32, tag="rb")
                    nc.tensor.matmul(out=rb_ps[:], lhsT=parT_t[:], rhs=rec[:],
                                     start=True, stop=True)
                    pi = spool.tile([P, JB], f32, tag="pi")
                    nc.vector.tensor_tensor(
                        out=pi[:], in0=E[:], in1=rb_ps[:], op=mybir.AluOpType.mult)

                    # ---- transposes to feature-major (4 blocks per psT bank)
                    HgT = wpool.tile([P, JB * DIM], f32, tag="hT")
                    RgT = wpool.tile([P, JB * DIM], f32, tag="rT")
                    TgT = wpool.tile([P, JB * DIM], f32, tag="tT")
                    for (src, dst, ei) in ((Hg, HgT, 0), (Rg, RgT, 1), (Tg, TgT, 2)):
                        for g in range(STB // 4):  # bank groups
                            tp = poolT.tile([P, 512], f32, tag="tp")
                            for q in range(4):
                                st = g * 4 + q
                                nc.tensor.transpose(
                                    out=tp[:, q * 128:(q + 1) * 128],
                                    in_=src[:, st * 128:(st + 1) * 128],
                                    identity=ident[:])
                            eng = nc.vector if (g + ei) % 2 == 0 else nc.scalar
                            if eng is nc.vector:
                                nc.vector.tensor_copy(
                                    out=dst[:, g * 512:(g + 1) * 512], in_=tp[:])
                            else:
                                nc.scalar.copy(
                                    out=dst[:, g * 512:(g + 1) * 512], in_=tp[:])

                    # ---- RNN step 1: A = Wh*H^T + Wr*R^T ; h1 = tanh(A + b2)
                    A_ps = poolR.tile([P, JB * DIM], f32, tag="rnn")
                    for st in range(STB):
                        nc.tensor.matmul(
                            out=A_ps[:, st * 128:(st + 1) * 128], lhsT=wh_t[:],
                            rhs=HgT[:, st * 128:(st + 1) * 128],
                            start=(st % 4 == 0), stop=False)
                    for st in range(STB):
                        nc.tensor.matmul(
                            out=A_ps[:, st * 128:(st + 1) * 128], lhsT=wr_t[:],
                            rhs=RgT[:, st * 128:(st + 1) * 128],
                            start=False, stop=(st % 4 == 3))
                    h1 = wpool.tile([P, JB * DIM], f32, tag="h1")
                    nc.scalar.activation(
                        out=h1[:], in_=A_ps[:],
                        func=mybir.ActivationFunctionType.Tanh, bias=b2_t[:, :])

                    # ---- RNN step 2: Bp = Wh*T^T + Wr*R^T + Whh*h1 ; h2T
                    B_ps = poolR.tile([P, JB * DIM], f32, tag="rnn")
                    for st in range(STB):
                        nc.tensor.matmul(
                            out=B_ps[:, st * 128:(st + 1) * 128], lhsT=wh_t[:],
                            rhs=TgT[:, st * 128:(st + 1) * 128],
                            start=(st % 4 == 0), stop=False)
                    for st in range(STB):
                        nc.tensor.matmul(
                            out=B_ps[:, st * 128:(st + 1) * 128], lhsT=wr_t[:],
                            rhs=RgT[:, st * 128:(st + 1) * 128],
                            start=False, stop=False)
                    for st in range(STB):
                        nc.tensor.matmul(
                            out=B_ps[:, st * 128:(st + 1) * 128], lhsT=whh_t[:],
                            rhs=h1[:, st * 128:(st + 1) * 128],
                            start=False, stop=(st % 4 == 3))
                    h2T = wpool.tile([P, JB * DIM], f32, tag="h2T")
                    nc.scalar.activation(
                        out=h2T[:], in_=B_ps[:],
                        func=mybir.ActivationFunctionType.Tanh, bias=b2_t[:, :])

                    # ---- back to token-major, scale by pi, accumulate into o
                    C_ps = poolR.tile([P, JB * DIM], f32, tag="rnn")
                    for st in range(STB):
                        nc.tensor.transpose(
                            out=C_ps[:, st * 128:(st + 1) * 128],
                            in_=h2T[:, st * 128:(st + 1) * 128], identity=ident[:])
                    scaled = wpool.tile([P, JB * DIM], f32, tag="sc")
                    for c in range(CPB):
                        nc.vector.tensor_tensor(
                            out=scaled[:, c * 512:(c + 1) * 512].rearrange(
                                "p (j d) -> p j d", d=DIM),
                            in0=C_ps[:, c * 512:(c + 1) * 512].rearrange(
                                "p (j d) -> p j d", d=DIM),
                            in1=pi[:, c * 16:(c + 1) * 16][:, :, None].to_broadcast(
                                [P, 16, DIM]),
                            op=mybir.AluOpType.mult)
                    last = (l == L - 1) and (b == NBATCH - 1)
                    for c in range(CPB):
                        o_accum(scaled[:, c * 512:(c + 1) * 512], b * CPB + c,
                                last and c == CPB - 1)

            # ---------------- final: sigmoid((o + ru[users]) . (e[items]+ri[items]))
            ru_g = spool.tile([NR, 512], f32, tag="ru")
            it_g = spool.tile([NR, 1024], f32, tag="itg")
            for jj in range(16):
                nc.gpsimd.indirect_dma_start(
                    out=ru_g[:, jj * DIM:(jj + 1) * DIM], out_offset=None,
                    in_=rec_user[:, :],
                    in_offset=bass.IndirectOffsetOnAxis(
                        ap=fu_t[:, jj:jj + 1], axis=0))
                nc.gpsimd.indirect_dma_start(
                    out=it_g[:, jj * 64:(jj + 1) * 64], out_offset=None,
                    in_=itab[:, :],
                    in_offset=bass.IndirectOffsetOnAxis(
                        ap=fi_t[:, jj:jj + 1], axis=0))
            ue = spool.tile([NR, 512], f32, tag="ue")
            nc.vector.tensor_tensor(out=ue[:], in0=o_ps[:], in1=ru_g[:],
                                    op=mybir.AluOpType.add)
            ie = spool.tile([NR, 512], f32, tag="ie2")
            itv = it_g[:].rearrange("p (j c d) -> p j c d", c=2, d=DIM)
            nc.vector.tensor_tensor(
                out=ie[:].rearrange("p (j d) -> p j d", d=DIM),
                in0=itv[:, :, 0, :], in1=itv[:, :, 1, :],
                op=mybir.AluOpType.add)
            pr = spool.tile([NR, 512], f32, tag="pr")
            nc.vector.tensor_tensor(out=pr[:], in0=ue[:], in1=ie[:],
                                    op=mybir.AluOpType.mult)
            sc = spool.tile([NR, 16], f32, tag="scs")
            nc.vector.tensor_reduce(
                out=sc[:], in_=pr[:].rearrange("p (j d) -> p j d", d=DIM),
                axis=mybir.AxisListType.X, op=mybir.AluOpType.add)
            sg = spool.tile([NR, 16], f32, tag="sg")
            nc.scalar.activation(out=sg[:], in_=sc[:],
                                 func=mybir.ActivationFunctionType.Sigmoid)
            nc.sync.dma_start(out=out_dram[:, :], in_=sg[:])

    nc.compile()
    return nc


# ---------------------------------------------------------------- host prep
_SHARED = {}


def _shared_tables(entity_emb, relation_emb, rec_item_emb):
    """Tables identical across cores: triple/pair relation products and the
    fused entity||rec_item table."""
    fp = (float(np.sum(relation_emb)), float(np.sum(rec_item_emb)),
          float(np.sum(entity_emb[:8])))
    if _SHARED.get("fp") != fp:
        _SHARED.clear()
        _SHARED["fp"] = fp
    if "rel3" not in _SHARED:
        e = np.asarray(relation_emb, np.float32)
        r3 = np.empty((N_RELATION, N_RELATION, N_RELATION, 3 * DIM), np.float32)
        r3[..., 0 * DIM:1 * DIM] = e[:, None, None, :]
        r3[..., 1 * DIM:2 * DIM] = e[None, :, None, :]
        r3[..., 2 * DIM:3 * DIM] = e[None, None, :, :]
        _SHARED["rel3"] = np.ascontiguousarray(r3.reshape(NREL3, 3 * DIM))
        rp = np.empty((N_RELATION, N_RELATION, 2 * DIM), np.float32)
        rp[..., 0 * DIM:1 * DIM] = e[:, None, :]
        rp[..., 1 * DIM:2 * DIM] = e[None, :, :]
        _SHARED["relp"] = np.ascontiguousarray(rp.reshape(NRELP, 2 * DIM))
        itab = np.empty((N_ITEM, 2 * DIM), np.float32)
        itab[:, :DIM] = np.asarray(entity_emb[:N_ITEM], np.float32)
        itab[:, DIM:] = np.asarray(rec_item_emb, np.float32)
        _SHARED["itab"] = itab
    return _SHARED


def _prep_core_inputs(c, BC, users, items, hop0_items, heads, relations, tails,
                      entity_emb, relation_emb, rec_user_emb, rec_item_emb,
                      W_ih, W_hh, b_ih, b_hh, JB=32):
    """numpy preprocessing: shard + index-layout permutations + const matrices."""
    J = BC // 2
    NBATCH = J // JB
    NCHUNK = J // 16
    NR = 2 * NCHUNK
    lo, hi = c * BC, (c + 1) * BC

    def glayout(a):  # [BC, K] -> [128, J]
        return np.ascontiguousarray(
            a.reshape(J, 2, K).transpose(1, 2, 0).reshape(P, J)).astype(np.int32)

    def flayout(a):  # [BC] -> [NR, 16]
        return np.ascontiguousarray(
            a.reshape(NCHUNK, 16, 2).transpose(0, 2, 1).reshape(NR, 16)).astype(np.int32)

    idx_h = np.concatenate([glayout(heads[l, lo:hi]) for l in range(L)], axis=1)
    idx_t = np.concatenate([glayout(tails[l, lo:hi]) for l in range(L)], axis=1)

    # relation triple/pair indices per (l, batch): batch covers j-columns
    # [b*JB, (b+1)*JB); first 30 go through rel3 in 10 triples, last 2
    # through relp.
    r3_cols = []
    rp_cols = []
    for l in range(L):
        r = glayout(relations[l, lo:hi])  # [128, J], values < 64
        for b in range(NBATCH):
            jlo = b * JB
            for jt in range(TRI):
                j0 = jlo + 3 * jt
                r3_cols.append(
                    r[:, j0] * (N_RELATION ** 2)
                    + r[:, j0 + 1] * N_RELATION + r[:, j0 + 2])
            rp_cols.append(r[:, jlo + 30] * N_RELATION + r[:, jlo + 31])
    idx_r3 = np.stack(r3_cols, axis=1).astype(np.int32)
    idx_rp = np.stack(rp_cols, axis=1).astype(np.int32)

    Wh = W_ih[:, :DIM]
    Wr = W_ih[:, DIM:]

    def blockdiag(w):  # w: [32, 32] block = w.T
        m = np.zeros((P, P), np.float32)
        for j in range(4):
            m[j * 32:(j + 1) * 32, j * 32:(j + 1) * 32] = w.T
        return m

    b2 = np.tile((b_ih + b_hh).astype(np.float32), 4)[:, None]

    sels = np.zeros((P, NCHUNK, NR), np.float32)
    pvec = np.arange(P) // 64  # parity of each partition
    for m in range(NCHUNK):
        for p in range(P):
            sels[p, m, 2 * m + pvec[p]] = 1.0
    par2 = np.zeros((P, 2), np.float32)
    par2[np.arange(P), pvec] = 1.0

    shared = _shared_tables(entity_emb, relation_emb, rec_item_emb)

    return {
        "entity": np.ascontiguousarray(entity_emb, np.float32),
        "rel3": shared["rel3"],
        "relp": shared["relp"],
        "rec_user": np.ascontiguousarray(rec_user_emb, np.float32),
        "itab": shared["itab"],
        "idx_hop0": glayout(hop0_items[lo:hi]),
        "idx_h": idx_h, "idx_t": idx_t,
        "idx_r3": idx_r3, "idx_rp": idx_rp,
        "fin_users": flayout(users[lo:hi]),
        "fin_items": flayout(items[lo:hi]),
        "wh_bd": blockdiag(Wh),
        "wr_bd": blockdiag(Wr),
        "whh_bd": blockdiag(W_hh),
        "b2": b2,
        "sels": np.ascontiguousarray(sels.reshape(P, NCHUNK * NR)),
        "par2": par2,
        "parT": np.ascontiguousarray(par2.T),
    }


def _unscramble(out_c, BC):
    """[NR, 16] core output -> [BC] user scores."""
    NCHUNK = (BC // 2) // 16
    return np.ascontiguousarray(
        out_c.reshape(NCHUNK, 2, 16).transpose(0, 2, 1).reshape(BC))


_CACHED = {}
TRACE = False  # set True (e.g. from test.py) to capture an NTFF profile
LAST_RESULTS = None


def kernel(**inputs):
    global LAST_RESULTS
    from concourse import bass_utils

    BC = B // NCORES
    if "nc" not in _CACHED:
        _CACHED["nc"] = build_core_program(BC=BC)
    nc = _CACHED["nc"]

    args = {k: np.asarray(v) for k, v in inputs.items()}
    in_maps = [
        _prep_core_inputs(
            c, BC,
            args["users"], args["items"], args["hop0_items"], args["heads"],
            args["relations"], args["tails"],
            np.asarray(args["entity_emb"], np.float32),
            np.asarray(args["relation_emb"], np.float32),
            np.asarray(args["rec_user_emb"], np.float32),
            np.asarray(args["rec_item_emb"], np.float32),
            np.asarray(args["W_ih"], np.float32),
            np.asarray(args["W_hh"], np.float32),
            np.asarray(args["b_ih"], np.float32),
            np.asarray(args["b_hh"], np.float32),
        )
        for c in range(NCORES)
    ]
    res = bass_utils.run_bass_kernel_spmd(
        nc, in_maps, core_ids=list(range(NCORES)), trace=TRACE)
    LAST_RESULTS = res
    out = np.concatenate(
        [_unscramble(res.results[c]["scores"], BC) for c in range(NCORES)])
    return out
